# revision 30
# baseline (speedup 1.0000x reference)
"""MoE expert-pool kernel for Trainium2, 8 NeuronCores.

Expert + tensor parallel with selective fp8:
  - Host: route tokens to experts (distinct (token,expert) pairs, combined
    routing weight cw per pair). Each expert keeps its highest-cw pairs in
    bf16 as exactly one <=512-token chunk (a partial tail chunk costs 256
    matmul issues regardless of width, so tails are never emitted); the
    rest run entirely in fp8-e4m3 DoubleRow matmuls. Strong caps (SA, SB)
    are chosen by scanning a measured cost model (bf16 chunk:
    256*(0.4167*tn+2.5) ns, fp8-DR chunk: 128*0.4226*tn ns) under a
    fitted error model err^2 = K2*weak_cw2_mass + C0SQ, capped at
    1.87e-2 (< 2e-2 gate; predictions match HW to 4 digits).
  - Experts are paired large+small; each core runs half-F slices of two
    experts (strong-a, strong-b, weak-a, weak-b phases), so per-core work
    is balanced. Half-F partial outputs are summed on the host.
  - Device per phase: yT_partial = W2h^T @ gelu(W1h^T @ XT + b1h), fp32
    PSUM, bf16 partial outputs. fp8 weight tiles reuse the bf16 weight
    SBUF buffers via tile-pool cycling. Outputs are partition-major
    [128, MT2*S] so quad DMAs use 4KB descriptors.
  - Timing guards: 64 warm matmuls bridge sequencer boot -> x0 arrival
    (HAM clock gate; a >=3.4us PE idle re-throttles to 1.2 GHz); the
    drain tail DMAs the last m-tiles individually with the final one in
    three shrinking pieces across both HWDGE queues (~2.7us drain).

Hardcoded problem shape: T=4096, H=1024, F=4096, E=8, K=2 (fp32 inputs).
"""

import sys
import types

import numpy as np
import ml_dtypes

H = 1024
F = 4096
FH = F // 2
E = 8
N_CORES = 8
PART = 128
TOK_CHUNK = 512  # fp32 PSUM bank = 512 columns

KT1 = H // PART    # 8  k-tiles for mm1 (contract over H)
MT1 = FH // PART   # 16 m-tiles for mm1 (output partitions = F-half chunks)
KT2 = FH // PART   # 16 k-tiles for mm2 (contract over F-half)
MT2 = H // PART    # 8  m-tiles for mm2 (output partitions = H chunks)

WARM_MMS = 64   # spans sequencer boot -> x0 arrival even on late-boot cores
W8SCALE = 256.0  # power-of-2 scale for fp8 weight quantization
ERR_CAP = 0.0187  # planned rel-err budget (gate is 2e-2)
# fitted on-HW error model: err^2 = K2*weak_mass + C0SQ (bf16-path floor)
K2_CAL = 9.7603e-7
C0SQ_CAL = 1.4303e-5
FIRST_CHUNK = None  # head chunk = full 512: smaller first chunks outrun the W1 DMA feed
# measured per-MM issue pace (ns, warm clock): strong bf16 chunk of tn
# tokens costs 256*(0.4167*tn+2.5), weak fp8-DR chunk costs 128*0.4226*tn
STRONG_NS = lambda tn: 256.0 * (0.4167 * tn + 2.5)
WEAK_NS = lambda tn: 128.0 * max(213.0, 0.4226 * tn)


def _install_axon_trace_shim():
    """Make run_bass_kernel_spmd(trace=True) survive images that lack
    antenv.axon_hooks (tracing degrades gracefully if the hook .so is
    unavailable)."""
    try:
        import antenv.axon_hooks  # noqa: F401
        return
    except ImportError:
        pass
    mod = types.ModuleType("antenv.axon_hooks")
    mod._hook = None

    def set_axon_ntff_profile_hook(h):
        mod._hook = h

    def get_axon_ntff_profile_hook():
        return mod._hook

    mod.set_axon_ntff_profile_hook = set_axon_ntff_profile_hook
    mod.get_axon_ntff_profile_hook = get_axon_ntff_profile_hook
    sys.modules["antenv.axon_hooks"] = mod
    try:
        import antenv
        antenv.axon_hooks = mod
    except ImportError:
        pass
    try:
        from trn_agent_boot.trn_boot import _ntff_profile_via_ctypes
        mod._hook = _ntff_profile_via_ctypes("/opt/axon/libaxon_pjrt.so")
    except Exception:
        pass


_install_axon_trace_shim()

_PROGRAM_CACHE = {}


def _chunks_of(C, first=None):
    chunks = []
    off = 0
    while off < C:
        n = min(TOK_CHUNK, C - off)
        if first is not None and off == 0:
            n = min(first, n)
        chunks.append((off, n))
        off += n
    return chunks


def _w1_groups():
    """W1-half DMA column groups: a small first group (one m-tile) so the
    first matmul group is gated by minimal bytes, then 512-wide groups."""
    groups = [(0, PART), (PART, 512 - PART)]
    groups += [(g, 512) for g in range(512, FH, 512)]
    return groups


def _w2_groups():
    return [(g, 512) for g in range(0, H, 512)]


def _pack_groups(w, kt, groups):
    """Pack a [kt*PART, cols] matrix into SBUF group-major layout
    [PART, kt*cols]: per group [p][(k, c)] contiguous."""
    w3 = w.reshape(kt, PART, w.shape[1])
    parts = [
        np.ascontiguousarray(
            w3[:, :, g0:g0 + gw].transpose(1, 0, 2).reshape(PART, kt * gw))
        for (g0, gw) in groups
    ]
    return np.ascontiguousarray(np.concatenate(parts, axis=1))


def _pack_3d(w, kt):
    """Pack [kt*PART, cols] into the [PART, kt, cols] device layout."""
    return np.ascontiguousarray(
        w.reshape(kt, PART, w.shape[1]).transpose(1, 0, 2))


def _build_program(SA, SB, WA, WB):
    """Per-core program: strong (bf16) + weak (fp8 DoubleRow) half-F jobs
    for two experts, token capacities SA/SB (strong) and WA/WB (weak)."""
    import concourse.mybir as mybir
    import concourse.tile as tile
    from concourse import bacc

    bf16 = mybir.dt.bfloat16
    f8 = mybir.dt.float8e4
    f32 = mybir.dt.float32
    DR = mybir.MatmulPerfMode.DoubleRow

    nc = bacc.Bacc("TRN2", target_bir_lowering=False, debug=False,
                   num_devices=N_CORES)

    w1_groups = _w1_groups()
    w2_groups = _w2_groups()

    xas_d = nc.dram_tensor("xas", [PART, KT1 * SA], bf16, kind="ExternalInput")
    xbs_d = nc.dram_tensor("xbs", [PART, KT1 * SB], bf16, kind="ExternalInput")
    xaw_d = nc.dram_tensor("xaw", [PART, KT1 * WA], f8, kind="ExternalInput")
    xbw_d = nc.dram_tensor("xbw", [PART, KT1 * WB], f8, kind="ExternalInput")
    w1a16_d = nc.dram_tensor("w1a16", [PART, KT1 * FH], bf16,
                             kind="ExternalInput")
    w1b16_d = nc.dram_tensor("w1b16", [PART, KT1 * FH], bf16,
                             kind="ExternalInput")
    w2a16_d = nc.dram_tensor("w2a16", [PART, KT2 * H], bf16,
                             kind="ExternalInput")
    w2b16_d = nc.dram_tensor("w2b16", [PART, KT2 * H], bf16,
                             kind="ExternalInput")
    w1a8_d = nc.dram_tensor("w1a8", [PART, KT1, FH], f8, kind="ExternalInput")
    w1b8_d = nc.dram_tensor("w1b8", [PART, KT1, FH], f8, kind="ExternalInput")
    w2a8_d = nc.dram_tensor("w2a8", [PART, KT2, H], f8, kind="ExternalInput")
    w2b8_d = nc.dram_tensor("w2b8", [PART, KT2, H], f8, kind="ExternalInput")
    b1ab_d = nc.dram_tensor("b1ab", [PART, 2 * MT1], f32,
                            kind="ExternalInput")
    # outputs are partition-major [PART, MT2*S] (m-blocks along the free
    # axis): a 4-m-tile quad then covers one contiguous 4*S-column run
    # per partition -- 128 descriptors of ~4KB instead of 512 of ~1KB,
    # which keeps the output stream from crowding the SDMA engines that
    # also carry the weight feed.  The host re-folds to [H, S].
    yas_d = nc.dram_tensor("yas", [PART, MT2 * SA], bf16,
                           kind="ExternalOutput")
    ybs_d = nc.dram_tensor("ybs", [PART, MT2 * SB], bf16,
                           kind="ExternalOutput")
    yaw_d = nc.dram_tensor("yaw", [PART, MT2 * WA], bf16,
                           kind="ExternalOutput")
    ybw_d = nc.dram_tensor("ybw", [PART, MT2 * WB], bf16,
                           kind="ExternalOutput")

    with tile.TileContext(nc) as tc:
        with (
            tc.tile_pool(name="xpool", bufs=1) as xpool,
            tc.tile_pool(name="wpool", bufs=4) as wpool,
            tc.tile_pool(name="hpool", bufs=1) as hpool,
            tc.tile_pool(name="consts", bufs=1) as consts,
            tc.tile_pool(name="stage", bufs=4) as stage_pool,
            tc.tile_pool(name="psum", bufs=7, space="PSUM") as psum_pool,
            tc.tile_pool(name="wpsum", bufs=1, space="PSUM") as wpsum_pool,
        ):
            gelu = mybir.ActivationFunctionType.Gelu

            # PE pre-warm: keeps the PE busy through the HAM activity
            # window so the real stream starts at 2.4 GHz.  memset on
            # gpsimd: it boots ~1us before the vector engine, so the
            # warm matmuls start as early as possible.
            warm_sb = consts.tile([PART, PART], bf16)
            nc.gpsimd.memset(warm_sb[:], 0.0)
            wps = wpsum_pool.tile([PART, PART], f32)
            for _ in range(WARM_MMS):
                nc.tensor.matmul(wps[:], warm_sb[:], warm_sb[:],
                                 start=True, stop=True)

            xas = xpool.tile([PART, KT1 * SA], bf16)
            xbs = xpool.tile([PART, KT1 * SB], bf16)
            xaw = xpool.tile([PART, KT1 * WA], f8)
            xbw = xpool.tile([PART, KT1 * WB], f8)
            b1ab = consts.tile([PART, 2 * MT1], f32)
            h_sb = hpool.tile([PART, MT1, TOK_CHUNK], bf16)
            h8_sb = hpool.tile([PART, KT2, TOK_CHUNK], f8)

            # Weight tiles cycle through 4 pool buffers: the fp8 tiles
            # (16KB/partition) reuse the bf16 buffers (32KB/partition)
            # after the strong phases release them.
            w1a16 = wpool.tile([PART, KT1 * FH], bf16, tag="w", name="w1a16")
            w2a16 = wpool.tile([PART, KT2 * H], bf16, tag="w", name="w2a16")
            w1b16 = wpool.tile([PART, KT1 * FH], bf16, tag="w", name="w1b16")
            w2b16 = wpool.tile([PART, KT2 * H], bf16, tag="w", name="w2b16")
            w1a8 = wpool.tile([PART, KT1, FH], f8, tag="w", name="w1a8")
            w2a8 = wpool.tile([PART, KT2, H], f8, tag="w", name="w2a8")
            w1b8 = wpool.tile([PART, KT1, FH], f8, tag="w", name="w1b8")
            w2b8 = wpool.tile([PART, KT2, H], f8, tag="w", name="w2b8")

            # --- DMA schedule.  Inputs on sync (its own 16 HW rings) in
            # consumption order; outputs + biases on scalar's rings.
            # Each dma_start pays ~1-3us of serial HWDGE descriptor-gen
            # per queue, so the head uses few, large transfers: w1's
            # first m-tile group + half of x0 on sync, the other x0 half
            # on scalar.  W1 groups must stay directly behind x0 on sync
            # or the first chunk's m-tiles starve (measured).
            (g0_, gw_) = w1_groups[0]
            nc.sync.dma_start(w1a16[:, g0_ * KT1:(g0_ + gw_) * KT1],
                              w1a16_d.ap()[:, g0_ * KT1:(g0_ + gw_) * KT1])
            sa_chunks = _chunks_of(SA, first=FIRST_CHUNK)
            (t00, tn0) = sa_chunks[0]
            # x0 split 3/5 across the queues: sync (serial behind w1g0)
            # carries less, and w1 group 1 rides scalar behind x0's
            # larger half so the m1..m3 weight feed doesn't queue behind
            # all of x0 on sync.
            for eng, (ka, kb) in ((nc.sync, (0, 3)),
                                  (nc.scalar, (3, KT1))):
                eng.dma_start(
                    xas[:, t00 * KT1 + ka * tn0:t00 * KT1 + kb * tn0],
                    xas_d.ap()[:, t00 * KT1 + ka * tn0:t00 * KT1 + kb * tn0])
            (g1_, gw1_) = w1_groups[1]
            nc.scalar.dma_start(w1a16[:, g1_ * KT1:(g1_ + gw1_) * KT1],
                                w1a16_d.ap()[:, g1_ * KT1:(g1_ + gw1_) * KT1])
            nc.scalar.dma_start(b1ab[:], b1ab_d.ap())
            for (g0, gw) in w1_groups[2:]:
                nc.sync.dma_start(w1a16[:, g0 * KT1:(g0 + gw) * KT1],
                                  w1a16_d.ap()[:, g0 * KT1:(g0 + gw) * KT1])
            for (t0, tn) in sa_chunks[1:]:
                nc.sync.dma_start(xas[:, t0 * KT1:(t0 + tn) * KT1],
                                  xas_d.ap()[:, t0 * KT1:(t0 + tn) * KT1])
            for (g0, gw) in w2_groups:
                nc.sync.dma_start(w2a16[:, g0 * KT2:(g0 + gw) * KT2],
                                  w2a16_d.ap()[:, g0 * KT2:(g0 + gw) * KT2])
            for (t0, tn) in _chunks_of(SB):
                nc.sync.dma_start(xbs[:, t0 * KT1:(t0 + tn) * KT1],
                                  xbs_d.ap()[:, t0 * KT1:(t0 + tn) * KT1])
            nc.sync.dma_start(w1b16[:], w1b16_d.ap())
            nc.sync.dma_start(w2b16[:], w2b16_d.ap())
            nc.sync.dma_start(xaw[:], xaw_d.ap())
            nc.sync.dma_start(xbw[:], xbw_d.ap())
            # fp8 weights: WAR-gated on the strong phases releasing bufs
            nc.sync.dma_start(w1a8[:], w1a8_d.ap())
            nc.sync.dma_start(w2a8[:], w2a8_d.ap())
            nc.sync.dma_start(w1b8[:], w1b8_d.ap())
            nc.sync.dma_start(w2b8[:], w2b8_d.ap())

            def x_slice(x_sb, t0, tn, k):
                base = t0 * KT1 + k * tn
                return x_sb[:, base:base + tn]

            def w_slice(w_sb, groups, kt, m, k):
                for (g0, gw) in groups:
                    if g0 <= m * PART < g0 + gw:
                        base = g0 * kt + k * gw + (m * PART - g0)
                        return w_sb[:, base:base + PART]
                raise AssertionError

            # Outputs are merged four m-tiles per dma_start (3D AP over
            # row-blocks) -- per-issue cost on the scalar sequencer is
            # ~1us, so per-m-tile issues can't keep up with the PE at
            # phase transitions.  qstate holds the in-flight quad tile.
            # For the final chunk of the final phase (the kernel's drain
            # tail) m-tiles past the first quad are DMA'd individually as
            # produced, and the very last one goes out in three pieces
            # alternating the two HWDGE queues (sync's input FIFO is
            # empty by then), so almost nothing is left in flight when
            # the PE finishes.
            qstate = {}

            def out_stage(ps, y_d, S, t0, tn, m, tail):
                q = m % 4
                base = m * S + t0
                if (t0 == 0 and tn == S) and (not tail or m < 4):
                    if q == 0:
                        qstate["tile"] = stage_pool.tile(
                            [PART, 4, TOK_CHUNK], bf16, tag="out", name="out")
                    qt = qstate["tile"]
                    nc.vector.tensor_copy(qt[:, q, :tn], ps[:, :tn])
                    if q == 3:
                        nc.scalar.dma_start(
                            y_d.ap()[:, (m - 3) * S:(m + 1) * S]
                            .rearrange("p (q t) -> p q t", q=4),
                            qt[:, :, :tn])
                    return
                out_sb = stage_pool.tile([PART, TOK_CHUNK], bf16,
                                         tag="tail", name="tail")
                if not (tail and m == MT2 - 1):
                    nc.vector.tensor_copy(out_sb[:, :tn], ps[:, :tn])
                    nc.scalar.dma_start(
                        y_d.ap()[:, base:base + tn], out_sb[:, :tn])
                    return
                # shrinking pieces: the last transfer (the only one that
                # cannot overlap compute) is small
                marks = [0, (2 * tn) // 5, (4 * tn) // 5, tn]
                cuts = list(zip(marks[:-1], marks[1:]))
                engs = (nc.scalar, nc.sync, nc.scalar)
                for eng, (a, b) in zip(engs, cuts):
                    nc.vector.tensor_copy(out_sb[:, a:b], ps[:, a:b])
                    eng.dma_start(
                        y_d.ap()[:, base + a:base + b], out_sb[:, a:b])

            def strong_phase(C, x_sb, w1_sb, w2_sb, b_off, y_d,
                             first=None):
                for (t0, tn) in _chunks_of(C, first=first):
                    for m in range(MT1):
                        ps = psum_pool.tile([PART, TOK_CHUNK], f32,
                                            tag="ps", name="ps")
                        for k in range(KT1):
                            nc.tensor.matmul(
                                ps[:, :tn],
                                w_slice(w1_sb, w1_groups, KT1, m, k),
                                x_slice(x_sb, t0, tn, k),
                                start=(k == 0), stop=(k == KT1 - 1))
                        nc.scalar.activation(
                            h_sb[:, m, :tn], ps[:, :tn], gelu,
                            bias=b1ab[:, b_off + m:b_off + m + 1],
                            scale=1.0)
                    for m in range(MT2):
                        ps = psum_pool.tile([PART, TOK_CHUNK], f32,
                                            tag="ps", name="ps")
                        for k in range(KT2):
                            nc.tensor.matmul(
                                ps[:, :tn],
                                w_slice(w2_sb, w2_groups, KT2, m, k),
                                h_sb[:, k, :tn],
                                start=(k == 0), stop=(k == KT2 - 1))
                        out_stage(ps, y_d, C, t0, tn, m, tail=False)

            def weak_phase(C, x8_sb, w18_sb, w28_sb, b_off, y_d, is_last):
                for (t0, tn) in _chunks_of(C):
                    for m in range(MT1):
                        ps = psum_pool.tile([PART, TOK_CHUNK], f32,
                                            tag="ps", name="ps")
                        for k in range(0, KT1, 2):
                            base = t0 * KT1 + k * tn
                            x3 = x8_sb[:, base:base + 2 * tn].rearrange(
                                "p (two t) -> p two t", two=2)
                            nc.tensor.matmul(
                                ps[:, :tn],
                                w18_sb[:, k:k + 2, m * PART:(m + 1) * PART],
                                x3,
                                start=(k == 0), stop=(k == KT1 - 2),
                                perf_mode=DR)
                        nc.scalar.activation(
                            h8_sb[:, m, :tn], ps[:, :tn], gelu,
                            bias=b1ab[:, b_off + m:b_off + m + 1],
                            scale=1.0 / W8SCALE)
                    tail = is_last and t0 + tn >= C
                    for m in range(MT2):
                        ps = psum_pool.tile([PART, TOK_CHUNK], f32,
                                            tag="ps", name="ps")
                        for k in range(0, KT2, 2):
                            nc.tensor.matmul(
                                ps[:, :tn],
                                w28_sb[:, k:k + 2, m * PART:(m + 1) * PART],
                                h8_sb[:, k:k + 2, :tn],
                                start=(k == 0), stop=(k == KT2 - 2),
                                perf_mode=DR)
                        out_stage(ps, y_d, C, t0, tn, m, tail=tail)

            strong_phase(SA, xas, w1a16, w2a16, 0, yas_d,
                         first=FIRST_CHUNK)
            strong_phase(SB, xbs, w1b16, w2b16, MT1, ybs_d)
            weak_phase(WA, xaw, w1a8, w2a8, 0, yaw_d, is_last=False)
            weak_phase(WB, xbw, w1b8, w2b8, MT1, ybw_d, is_last=True)

    nc.compile()
    return nc


def _route(expert_weights, selected_experts):
    """Distinct (token, expert) pairs with combined weights."""
    se = np.asarray(selected_experts).astype(np.int64)
    ew = np.asarray(expert_weights).astype(np.float32)
    routes = []
    for e in range(E):
        hit = (se == e)  # [T, K]
        tok = np.nonzero(hit.any(axis=1))[0]
        cw = (ew * hit).sum(axis=1)[tok]
        routes.append((tok, cw))
    return routes


def _plan_split(routes):
    """Choose per-expert fp8 (weak) quotas and expert pairing.

    The device stream cost is chunk-quantized: a strong (bf16) chunk of
    tn<=512 tokens costs 256 matmul issues regardless of tn (LDW/issue
    floor), so partial tail chunks are pure waste.  Cap strong streams at
    exactly one full chunk (512) and push the remainder (each expert's
    lowest-cw tokens) through the fp8 weak path, which must itself stay
    <= 512 tokens per expert (single DR chunk).  The weak cw^2 mass sets
    the output error: err ~= K_CAL * sqrt(mass); grow the strong caps
    (re-admitting tail chunks) only if the budget would be exceeded."""
    C = np.array([len(t) for t, _ in routes])
    pref = []
    for _, cw in routes:
        s = np.sort(cw)
        pref.append(np.concatenate([[0.0], np.cumsum(s ** 2)]))

    order = np.argsort(-C, kind="stable")
    arole = [int(e) for e in order[:4]]
    brole = [int(e) for e in order[4:]]

    def mass(SA, SB):
        return (sum(pref[e][max(0, int(C[e]) - SA)] for e in arole)
                + sum(pref[e][max(0, int(C[e]) - SB)] for e in brole))

    def err(SA, SB):
        return float(np.sqrt(K2_CAL * mass(SA, SB) + C0SQ_CAL))

    def stream_cost(S, weak):
        per_chunk = WEAK_NS if weak else STRONG_NS
        n_mm_floor = 128 * 31.0 if weak else 256 * 31.0
        return sum(max(per_chunk(tn), n_mm_floor)
                   for (_, tn) in _chunks_of(max(S, 1)))

    def plan_cost(SA, SB):
        WA = _round16(max(int(C[arole].max()) - SA, 16))
        WB = _round16(max(int(C[brole].max()) - SB, 16))
        return (stream_cost(SA, False) + stream_cost(SB, False)
                + stream_cost(WA, True) + stream_cost(WB, True))

    maxA, maxB = int(C[arole].max()), int(C[brole].max())
    best = None
    for SA in range(TOK_CHUNK, max(TOK_CHUNK - 129, maxA - TOK_CHUNK - 1), -16):
        for SB in range(TOK_CHUNK, max(TOK_CHUNK - 129, maxB - TOK_CHUNK - 1), -16):
            if maxA - SA > TOK_CHUNK or maxB - SB > TOK_CHUNK:
                continue  # weak stream must stay a single DR chunk
            if err(SA, SB) > ERR_CAP:
                continue
            c = plan_cost(SA, SB)
            if best is None or c < best[0]:
                best = (c, SA, SB)
    if best is None:
        # error-budget safety valve: nothing feasible under the cap, so
        # run every pair in bf16 (correct, just slower: tail chunks)
        SA, SB = maxA, maxB
    else:
        _, SA, SB = best

    split = []
    for e, (tok, cw) in enumerate(routes):
        cap = SA if e in arole else SB
        w_e = max(0, len(tok) - cap)
        idx = np.argsort(cw, kind="stable")
        wk = np.zeros(len(tok), dtype=bool)
        wk[idx[:w_e]] = True
        split.append(((tok[~wk], cw[~wk]), (tok[wk], cw[wk])))
    pairs = [(a, b) for a, b in zip(arole, brole)]
    return split, pairs


def _round16(n):
    return max(16, (n + 15) // 16 * 16)


def _pack_x_bf16(hs, tok, C, first=None):
    bf16 = ml_dtypes.bfloat16
    xt = np.zeros((H, C), dtype=bf16)
    if len(tok):
        xt[:, :len(tok)] = hs[tok].T.astype(bf16)
    return _pack_groups(xt, KT1, _chunks_of(C, first=first))


def _pack_x_f8(hs, tok, C):
    f8 = ml_dtypes.float8_e4m3
    xt = np.zeros((H, C), dtype=f8)
    if len(tok):
        xt[:, :len(tok)] = np.clip(hs[tok].T, -240, 240).astype(f8)
    return _pack_groups(xt, KT1, _chunks_of(C))


def _q8(w):
    return np.clip(w * W8SCALE, -240, 240).astype(ml_dtypes.float8_e4m3)


def kernel(hidden_states, expert_weights, W1, b1, W2, b2, selected_experts):
    from concourse.bass_utils import run_bass_kernel_spmd

    hs = np.asarray(hidden_states)
    out_dtype = hs.dtype
    hs = hs.astype(np.float32)
    W1 = np.asarray(W1).astype(np.float32)
    b1 = np.asarray(b1).astype(np.float32)
    W2 = np.asarray(W2).astype(np.float32)
    b2 = np.asarray(b2).astype(np.float32)

    T = hs.shape[0]
    assert hs.shape[1] == H and W1.shape == (E, H, F) and W2.shape == (E, F, H)

    routes = _route(expert_weights, selected_experts)
    split, pairs = _plan_split(routes)
    s_cnt = np.array([len(s[0][0]) for s in split])
    w_cnt = np.array([len(s[1][0]) for s in split])

    SA = max(PART, int(max(s_cnt[a] for a, _ in pairs)))
    SB = max(PART, int(max(s_cnt[b] for _, b in pairs)))
    WA = _round16(int(max(w_cnt[a] for a, _ in pairs)))
    WB = _round16(int(max(w_cnt[b] for _, b in pairs)))

    key = (SA, SB, WA, WB)
    if key not in _PROGRAM_CACHE:
        _PROGRAM_CACHE[key] = _build_program(*key)
    nc = _PROGRAM_CACHE[key]

    bf16 = ml_dtypes.bfloat16
    w1_groups = _w1_groups()
    w2_groups = _w2_groups()

    w1h16 = {}
    w2h16 = {}
    w1h8 = {}
    w2h8 = {}
    b1h = {}
    for e in set(e for p in pairs for e in p):
        for half in (0, 1):
            c0, c1 = half * FH, (half + 1) * FH
            w1e = W1[e][:, c0:c1]
            w2e = W2[e][c0:c1, :]
            w1h16[(e, half)] = _pack_groups(w1e.astype(bf16), KT1, w1_groups)
            w2h16[(e, half)] = _pack_groups(w2e.astype(bf16), KT2, w2_groups)
            w1h8[(e, half)] = _pack_3d(_q8(w1e), KT1)
            w2h8[(e, half)] = _pack_3d(_q8(w2e), KT2)
            b1h[(e, half)] = np.ascontiguousarray(
                b1[e][c0:c1].reshape(MT1, PART).T)

    xs_cache = {}
    xw_cache = {}
    in_maps = []
    for (a, b) in pairs:
        if a not in xs_cache:
            xs_cache[a] = _pack_x_bf16(hs, split[a][0][0], SA,
                                       first=FIRST_CHUNK)
            xw_cache[a] = _pack_x_f8(hs, split[a][1][0], WA)
        if b not in xs_cache:
            xs_cache[b] = _pack_x_bf16(hs, split[b][0][0], SB)
            xw_cache[b] = _pack_x_f8(hs, split[b][1][0], WB)
        for half in (0, 1):
            in_maps.append({
                "xas": xs_cache[a], "xaw": xw_cache[a],
                "xbs": xs_cache[b], "xbw": xw_cache[b],
                "w1a16": w1h16[(a, half)], "w2a16": w2h16[(a, half)],
                "w1a8": w1h8[(a, half)], "w2a8": w2h8[(a, half)],
                "w1b16": w1h16[(b, 1 - half)], "w2b16": w2h16[(b, 1 - half)],
                "w1b8": w1h8[(b, 1 - half)], "w2b8": w2h8[(b, 1 - half)],
                "b1ab": np.ascontiguousarray(np.concatenate(
                    [b1h[(a, half)], b1h[(b, 1 - half)]], axis=1)),
            })

    res = run_bass_kernel_spmd(nc, in_maps, core_ids=list(range(N_CORES)))

    def unfold(buf, cnt):
        # [PART, MT2*S] partition-major device layout -> [H, cnt]
        S = buf.shape[1] // MT2
        return (buf.reshape(PART, MT2, S).transpose(1, 0, 2)
                .reshape(H, S)[:, :cnt].astype(np.float32))

    out = np.zeros((T, H), dtype=np.float32)
    for pi, (a, b) in enumerate(pairs):
        r0 = res.results[2 * pi]
        r1 = res.results[2 * pi + 1]
        for e, skey, wkey in ((a, "yas", "yaw"), (b, "ybs", "ybw")):
            (tok_s, cw_s), (tok_w, cw_w) = split[e]
            if len(tok_s):
                yt = (unfold(r0[skey], len(tok_s))
                      + unfold(r1[skey], len(tok_s)))
                out[tok_s] += cw_s[:, None] * (yt.T + b2[e][None, :])
            if len(tok_w):
                yt = (unfold(r0[wkey], len(tok_w))
                      + unfold(r1[wkey], len(tok_w)))
                out[tok_w] += cw_w[:, None] * (yt.T / W8SCALE
                                               + b2[e][None, :])
    return out.astype(out_dtype)



# revision 31
# speedup vs baseline: 1.0167x; 1.0167x over previous
"""MoE expert-pool kernel for Trainium2, 8 NeuronCores.

Expert + tensor parallel with selective fp8:
  - Host: route tokens to experts (distinct (token,expert) pairs, combined
    routing weight cw per pair). Each expert keeps its highest-cw pairs in
    bf16 as exactly one <=512-token chunk (a partial tail chunk costs 256
    matmul issues regardless of width, so tails are never emitted); the
    rest run entirely in fp8-e4m3 DoubleRow matmuls. Strong caps (SA, SB)
    are chosen by scanning a measured cost model (bf16 chunk:
    256*(0.4167*tn+2.5) ns, fp8-DR chunk: 128*0.4226*tn ns) under a
    fitted error model err^2 = K2*weak_cw2_mass + C0SQ, capped at
    1.87e-2 (< 2e-2 gate; predictions match HW to 4 digits).
  - Experts are paired large+small; each core runs half-F slices of two
    experts (strong-a, strong-b, weak-a, weak-b phases), so per-core work
    is balanced. Half-F partial outputs are summed on the host.
  - Device per phase: yT_partial = W2h^T @ gelu(W1h^T @ XT + b1h), fp32
    PSUM, bf16 partial outputs. fp8 weight tiles reuse the bf16 weight
    SBUF buffers via tile-pool cycling. Outputs are partition-major
    [128, MT2*S] so quad DMAs use 4KB descriptors.
  - Timing guards: 64 warm matmuls bridge sequencer boot -> x0 arrival
    (HAM clock gate; a >=3.4us PE idle re-throttles to 1.2 GHz); the
    drain tail DMAs the last m-tiles individually with the final one in
    three shrinking pieces across both HWDGE queues (~2.7us drain).

Hardcoded problem shape: T=4096, H=1024, F=4096, E=8, K=2 (fp32 inputs).
"""

import sys
import types

import numpy as np
import ml_dtypes

H = 1024
F = 4096
FH = F // 2
E = 8
N_CORES = 8
PART = 128
TOK_CHUNK = 512  # fp32 PSUM bank = 512 columns

KT1 = H // PART    # 8  k-tiles for mm1 (contract over H)
MT1 = FH // PART   # 16 m-tiles for mm1 (output partitions = F-half chunks)
KT2 = FH // PART   # 16 k-tiles for mm2 (contract over F-half)
MT2 = H // PART    # 8  m-tiles for mm2 (output partitions = H chunks)

WARM_MMS = 64   # spans sequencer boot -> x0 arrival even on late-boot cores
W8SCALE = 256.0  # power-of-2 scale for fp8 weight quantization
ERR_CAP = 0.0187  # planned rel-err budget (gate is 2e-2)
# fitted on-HW error model: err^2 = K2*weak_mass + C0SQ (bf16-path floor)
K2_CAL = 9.7603e-7
C0SQ_CAL = 1.4303e-5
FIRST_CHUNK = None  # head chunk = full 512: smaller first chunks outrun the W1 DMA feed
# measured per-MM issue pace (ns, warm clock): strong bf16 chunk of tn
# tokens costs 256*(0.4167*tn+2.5), weak fp8-DR chunk costs 128*0.4226*tn
STRONG_NS = lambda tn: 256.0 * (0.4167 * tn + 2.5)
WEAK_NS = lambda tn: 128.0 * max(213.0, 0.4226 * tn)


def _install_axon_trace_shim():
    """Make run_bass_kernel_spmd(trace=True) survive images that lack
    antenv.axon_hooks (tracing degrades gracefully if the hook .so is
    unavailable)."""
    try:
        import antenv.axon_hooks  # noqa: F401
        return
    except ImportError:
        pass
    mod = types.ModuleType("antenv.axon_hooks")
    mod._hook = None

    def set_axon_ntff_profile_hook(h):
        mod._hook = h

    def get_axon_ntff_profile_hook():
        return mod._hook

    mod.set_axon_ntff_profile_hook = set_axon_ntff_profile_hook
    mod.get_axon_ntff_profile_hook = get_axon_ntff_profile_hook
    sys.modules["antenv.axon_hooks"] = mod
    try:
        import antenv
        antenv.axon_hooks = mod
    except ImportError:
        pass
    try:
        from trn_agent_boot.trn_boot import _ntff_profile_via_ctypes
        mod._hook = _ntff_profile_via_ctypes("/opt/axon/libaxon_pjrt.so")
    except Exception:
        pass


_install_axon_trace_shim()

_PROGRAM_CACHE = {}


def _chunks_of(C, first=None):
    chunks = []
    off = 0
    while off < C:
        n = min(TOK_CHUNK, C - off)
        if first is not None and off == 0:
            n = min(first, n)
        chunks.append((off, n))
        off += n
    return chunks


def _w1_groups():
    """W1-half DMA column groups: a small first group (one m-tile) so the
    first matmul group is gated by minimal bytes, then 512-wide groups."""
    groups = [(0, PART), (PART, 512 - PART)]
    groups += [(g, 512) for g in range(512, FH, 512)]
    return groups


def _w2_groups():
    return [(g, 512) for g in range(0, H, 512)]


def _pack_groups(w, kt, groups):
    """Pack a [kt*PART, cols] matrix into SBUF group-major layout
    [PART, kt*cols]: per group [p][(k, c)] contiguous."""
    w3 = w.reshape(kt, PART, w.shape[1])
    parts = [
        np.ascontiguousarray(
            w3[:, :, g0:g0 + gw].transpose(1, 0, 2).reshape(PART, kt * gw))
        for (g0, gw) in groups
    ]
    return np.ascontiguousarray(np.concatenate(parts, axis=1))


def _pack_3d(w, kt):
    """Pack [kt*PART, cols] into the [PART, kt, cols] device layout."""
    return np.ascontiguousarray(
        w.reshape(kt, PART, w.shape[1]).transpose(1, 0, 2))


def _build_program(SA, SB, WA, WB):
    """Per-core program: strong (bf16) + weak (fp8 DoubleRow) half-F jobs
    for two experts, token capacities SA/SB (strong) and WA/WB (weak)."""
    import concourse.mybir as mybir
    import concourse.tile as tile
    from concourse import bacc

    bf16 = mybir.dt.bfloat16
    f8 = mybir.dt.float8e4
    f32 = mybir.dt.float32
    DR = mybir.MatmulPerfMode.DoubleRow

    nc = bacc.Bacc("TRN2", target_bir_lowering=False, debug=False,
                   num_devices=N_CORES)

    w1_groups = _w1_groups()
    w2_groups = _w2_groups()

    xas_d = nc.dram_tensor("xas", [PART, KT1 * SA], bf16, kind="ExternalInput")
    xbs_d = nc.dram_tensor("xbs", [PART, KT1 * SB], bf16, kind="ExternalInput")
    xaw_d = nc.dram_tensor("xaw", [PART, KT1 * WA], f8, kind="ExternalInput")
    xbw_d = nc.dram_tensor("xbw", [PART, KT1 * WB], f8, kind="ExternalInput")
    w1a16_d = nc.dram_tensor("w1a16", [PART, KT1 * FH], bf16,
                             kind="ExternalInput")
    w1b16_d = nc.dram_tensor("w1b16", [PART, KT1 * FH], bf16,
                             kind="ExternalInput")
    w2a16_d = nc.dram_tensor("w2a16", [PART, KT2 * H], bf16,
                             kind="ExternalInput")
    w2b16_d = nc.dram_tensor("w2b16", [PART, KT2 * H], bf16,
                             kind="ExternalInput")
    w1a8_d = nc.dram_tensor("w1a8", [PART, KT1, FH], f8, kind="ExternalInput")
    w1b8_d = nc.dram_tensor("w1b8", [PART, KT1, FH], f8, kind="ExternalInput")
    w2a8_d = nc.dram_tensor("w2a8", [PART, KT2, H], f8, kind="ExternalInput")
    w2b8_d = nc.dram_tensor("w2b8", [PART, KT2, H], f8, kind="ExternalInput")
    b1ab_d = nc.dram_tensor("b1ab", [PART, 2 * MT1], f32,
                            kind="ExternalInput")
    # outputs are partition-major [PART, MT2*S] (m-blocks along the free
    # axis): a 4-m-tile quad then covers one contiguous 4*S-column run
    # per partition -- 128 descriptors of ~4KB instead of 512 of ~1KB,
    # which keeps the output stream from crowding the SDMA engines that
    # also carry the weight feed.  The host re-folds to [H, S].
    yas_d = nc.dram_tensor("yas", [PART, MT2 * SA], bf16,
                           kind="ExternalOutput")
    ybs_d = nc.dram_tensor("ybs", [PART, MT2 * SB], bf16,
                           kind="ExternalOutput")
    yaw_d = nc.dram_tensor("yaw", [PART, MT2 * WA], bf16,
                           kind="ExternalOutput")
    ybw_d = nc.dram_tensor("ybw", [PART, MT2 * WB], bf16,
                           kind="ExternalOutput")

    with tile.TileContext(nc) as tc:
        with (
            tc.tile_pool(name="xpool", bufs=1) as xpool,
            tc.tile_pool(name="wpool", bufs=4) as wpool,
            tc.tile_pool(name="hpool", bufs=1) as hpool,
            tc.tile_pool(name="consts", bufs=1) as consts,
            tc.tile_pool(name="stage", bufs=4) as stage_pool,
            tc.tile_pool(name="psum", bufs=7, space="PSUM") as psum_pool,
            tc.tile_pool(name="wpsum", bufs=1, space="PSUM") as wpsum_pool,
        ):
            gelu = mybir.ActivationFunctionType.Gelu

            # PE pre-warm: keeps the PE busy through the HAM activity
            # window so the real stream starts at 2.4 GHz.  memset on
            # gpsimd: it boots ~1us before the vector engine, so the
            # warm matmuls start as early as possible.
            warm_sb = consts.tile([PART, PART], bf16)
            nc.gpsimd.memset(warm_sb[:], 0.0)
            wps = wpsum_pool.tile([PART, PART], f32)
            for _ in range(WARM_MMS):
                nc.tensor.matmul(wps[:], warm_sb[:], warm_sb[:],
                                 start=True, stop=True)

            xas = xpool.tile([PART, KT1 * SA], bf16)
            xbs = xpool.tile([PART, KT1 * SB], bf16)
            xaw = xpool.tile([PART, KT1 * WA], f8)
            xbw = xpool.tile([PART, KT1 * WB], f8)
            b1ab = consts.tile([PART, 2 * MT1], f32)
            h_sb = hpool.tile([PART, MT1, TOK_CHUNK], bf16)
            h8_sb = hpool.tile([PART, KT2, TOK_CHUNK], f8)

            # Weight tiles cycle through 4 pool buffers: the fp8 tiles
            # (16KB/partition) reuse the bf16 buffers (32KB/partition)
            # after the strong phases release them.
            w1a16 = wpool.tile([PART, KT1 * FH], bf16, tag="w", name="w1a16")
            w2a16 = wpool.tile([PART, KT2 * H], bf16, tag="w", name="w2a16")
            w1b16 = wpool.tile([PART, KT1 * FH], bf16, tag="w", name="w1b16")
            w2b16 = wpool.tile([PART, KT2 * H], bf16, tag="w", name="w2b16")
            w1a8 = wpool.tile([PART, KT1, FH], f8, tag="w", name="w1a8")
            w2a8 = wpool.tile([PART, KT2, H], f8, tag="w", name="w2a8")
            w1b8 = wpool.tile([PART, KT1, FH], f8, tag="w", name="w1b8")
            w2b8 = wpool.tile([PART, KT2, H], f8, tag="w", name="w2b8")

            # --- DMA schedule.  Inputs on sync (its own 16 HW rings) in
            # consumption order; outputs + biases on scalar's rings.
            # Each dma_start pays ~1-3us of serial HWDGE descriptor-gen
            # per queue, so the head uses few, large transfers: w1's
            # first m-tile group + half of x0 on sync, the other x0 half
            # on scalar.  W1 groups must stay directly behind x0 on sync
            # or the first chunk's m-tiles starve (measured).
            (g0_, gw_) = w1_groups[0]
            nc.sync.dma_start(w1a16[:, g0_ * KT1:(g0_ + gw_) * KT1],
                              w1a16_d.ap()[:, g0_ * KT1:(g0_ + gw_) * KT1])
            sa_chunks = _chunks_of(SA, first=FIRST_CHUNK)
            (t00, tn0) = sa_chunks[0]
            half_k = KT1 // 2
            for eng, (ka, kb) in ((nc.sync, (0, half_k)),
                                  (nc.scalar, (half_k, KT1))):
                eng.dma_start(
                    xas[:, t00 * KT1 + ka * tn0:t00 * KT1 + kb * tn0],
                    xas_d.ap()[:, t00 * KT1 + ka * tn0:t00 * KT1 + kb * tn0])
            nc.scalar.dma_start(b1ab[:], b1ab_d.ap())
            for (g0, gw) in w1_groups[1:]:
                nc.sync.dma_start(w1a16[:, g0 * KT1:(g0 + gw) * KT1],
                                  w1a16_d.ap()[:, g0 * KT1:(g0 + gw) * KT1])
            for (t0, tn) in sa_chunks[1:]:
                nc.sync.dma_start(xas[:, t0 * KT1:(t0 + tn) * KT1],
                                  xas_d.ap()[:, t0 * KT1:(t0 + tn) * KT1])
            for (g0, gw) in w2_groups:
                nc.sync.dma_start(w2a16[:, g0 * KT2:(g0 + gw) * KT2],
                                  w2a16_d.ap()[:, g0 * KT2:(g0 + gw) * KT2])
            for (t0, tn) in _chunks_of(SB):
                nc.sync.dma_start(xbs[:, t0 * KT1:(t0 + tn) * KT1],
                                  xbs_d.ap()[:, t0 * KT1:(t0 + tn) * KT1])
            nc.sync.dma_start(w1b16[:], w1b16_d.ap())
            nc.sync.dma_start(w2b16[:], w2b16_d.ap())
            nc.sync.dma_start(xaw[:], xaw_d.ap())
            nc.sync.dma_start(xbw[:], xbw_d.ap())
            # fp8 weights: WAR-gated on the strong phases releasing bufs
            nc.sync.dma_start(w1a8[:], w1a8_d.ap())
            nc.sync.dma_start(w2a8[:], w2a8_d.ap())
            nc.sync.dma_start(w1b8[:], w1b8_d.ap())
            nc.sync.dma_start(w2b8[:], w2b8_d.ap())

            def x_slice(x_sb, t0, tn, k):
                base = t0 * KT1 + k * tn
                return x_sb[:, base:base + tn]

            def w_slice(w_sb, groups, kt, m, k):
                for (g0, gw) in groups:
                    if g0 <= m * PART < g0 + gw:
                        base = g0 * kt + k * gw + (m * PART - g0)
                        return w_sb[:, base:base + PART]
                raise AssertionError

            # Outputs are merged four m-tiles per dma_start (3D AP over
            # row-blocks) -- per-issue cost on the scalar sequencer is
            # ~1us, so per-m-tile issues can't keep up with the PE at
            # phase transitions.  qstate holds the in-flight quad tile.
            # For the final chunk of the final phase (the kernel's drain
            # tail) m-tiles past the first quad are DMA'd individually as
            # produced, and the very last one goes out in three pieces
            # alternating the two HWDGE queues (sync's input FIFO is
            # empty by then), so almost nothing is left in flight when
            # the PE finishes.
            qstate = {}

            def out_stage(ps, y_d, S, t0, tn, m, tail):
                q = m % 4
                base = m * S + t0
                if (t0 == 0 and tn == S) and (not tail or m < 4):
                    if q == 0:
                        qstate["tile"] = stage_pool.tile(
                            [PART, 4, TOK_CHUNK], bf16, tag="out", name="out")
                    qt = qstate["tile"]
                    nc.vector.tensor_copy(qt[:, q, :tn], ps[:, :tn])
                    if q == 3:
                        nc.scalar.dma_start(
                            y_d.ap()[:, (m - 3) * S:(m + 1) * S]
                            .rearrange("p (q t) -> p q t", q=4),
                            qt[:, :, :tn])
                    return
                out_sb = stage_pool.tile([PART, TOK_CHUNK], bf16,
                                         tag="tail", name="tail")
                if not (tail and m == MT2 - 1):
                    nc.vector.tensor_copy(out_sb[:, :tn], ps[:, :tn])
                    nc.scalar.dma_start(
                        y_d.ap()[:, base:base + tn], out_sb[:, :tn])
                    return
                # shrinking pieces: the last transfer (the only one that
                # cannot overlap compute) is small
                marks = [0, (2 * tn) // 5, (4 * tn) // 5, tn]
                cuts = list(zip(marks[:-1], marks[1:]))
                engs = (nc.scalar, nc.sync, nc.scalar)
                for eng, (a, b) in zip(engs, cuts):
                    nc.vector.tensor_copy(out_sb[:, a:b], ps[:, a:b])
                    eng.dma_start(
                        y_d.ap()[:, base + a:base + b], out_sb[:, a:b])

            def strong_phase(C, x_sb, w1_sb, w2_sb, b_off, y_d,
                             first=None):
                for (t0, tn) in _chunks_of(C, first=first):
                    for m in range(MT1):
                        ps = psum_pool.tile([PART, TOK_CHUNK], f32,
                                            tag="ps", name="ps")
                        for k in range(KT1):
                            nc.tensor.matmul(
                                ps[:, :tn],
                                w_slice(w1_sb, w1_groups, KT1, m, k),
                                x_slice(x_sb, t0, tn, k),
                                start=(k == 0), stop=(k == KT1 - 1))
                        nc.scalar.activation(
                            h_sb[:, m, :tn], ps[:, :tn], gelu,
                            bias=b1ab[:, b_off + m:b_off + m + 1],
                            scale=1.0)
                    for m in range(MT2):
                        ps = psum_pool.tile([PART, TOK_CHUNK], f32,
                                            tag="ps", name="ps")
                        for k in range(KT2):
                            nc.tensor.matmul(
                                ps[:, :tn],
                                w_slice(w2_sb, w2_groups, KT2, m, k),
                                h_sb[:, k, :tn],
                                start=(k == 0), stop=(k == KT2 - 1))
                        out_stage(ps, y_d, C, t0, tn, m, tail=False)

            def weak_phase(C, x8_sb, w18_sb, w28_sb, b_off, y_d, is_last):
                for (t0, tn) in _chunks_of(C):
                    for m in range(MT1):
                        ps = psum_pool.tile([PART, TOK_CHUNK], f32,
                                            tag="ps", name="ps")
                        for k in range(0, KT1, 2):
                            base = t0 * KT1 + k * tn
                            x3 = x8_sb[:, base:base + 2 * tn].rearrange(
                                "p (two t) -> p two t", two=2)
                            nc.tensor.matmul(
                                ps[:, :tn],
                                w18_sb[:, k:k + 2, m * PART:(m + 1) * PART],
                                x3,
                                start=(k == 0), stop=(k == KT1 - 2),
                                perf_mode=DR)
                        nc.scalar.activation(
                            h8_sb[:, m, :tn], ps[:, :tn], gelu,
                            bias=b1ab[:, b_off + m:b_off + m + 1],
                            scale=1.0 / W8SCALE)
                    tail = is_last and t0 + tn >= C
                    for m in range(MT2):
                        ps = psum_pool.tile([PART, TOK_CHUNK], f32,
                                            tag="ps", name="ps")
                        for k in range(0, KT2, 2):
                            nc.tensor.matmul(
                                ps[:, :tn],
                                w28_sb[:, k:k + 2, m * PART:(m + 1) * PART],
                                h8_sb[:, k:k + 2, :tn],
                                start=(k == 0), stop=(k == KT2 - 2),
                                perf_mode=DR)
                        out_stage(ps, y_d, C, t0, tn, m, tail=tail)

            strong_phase(SA, xas, w1a16, w2a16, 0, yas_d,
                         first=FIRST_CHUNK)
            strong_phase(SB, xbs, w1b16, w2b16, MT1, ybs_d)
            weak_phase(WA, xaw, w1a8, w2a8, 0, yaw_d, is_last=False)
            weak_phase(WB, xbw, w1b8, w2b8, MT1, ybw_d, is_last=True)

    nc.compile()
    return nc


def _route(expert_weights, selected_experts):
    """Distinct (token, expert) pairs with combined weights."""
    se = np.asarray(selected_experts).astype(np.int64)
    ew = np.asarray(expert_weights).astype(np.float32)
    routes = []
    for e in range(E):
        hit = (se == e)  # [T, K]
        tok = np.nonzero(hit.any(axis=1))[0]
        cw = (ew * hit).sum(axis=1)[tok]
        routes.append((tok, cw))
    return routes


def _plan_split(routes):
    """Choose per-expert fp8 (weak) quotas and expert pairing.

    The device stream cost is chunk-quantized: a strong (bf16) chunk of
    tn<=512 tokens costs 256 matmul issues regardless of tn (LDW/issue
    floor), so partial tail chunks are pure waste.  Cap strong streams at
    exactly one full chunk (512) and push the remainder (each expert's
    lowest-cw tokens) through the fp8 weak path, which must itself stay
    <= 512 tokens per expert (single DR chunk).  The weak cw^2 mass sets
    the output error: err ~= K_CAL * sqrt(mass); grow the strong caps
    (re-admitting tail chunks) only if the budget would be exceeded."""
    C = np.array([len(t) for t, _ in routes])
    pref = []
    for _, cw in routes:
        s = np.sort(cw)
        pref.append(np.concatenate([[0.0], np.cumsum(s ** 2)]))

    order = np.argsort(-C, kind="stable")
    arole = [int(e) for e in order[:4]]
    brole = [int(e) for e in order[4:]]

    def mass(SA, SB):
        return (sum(pref[e][max(0, int(C[e]) - SA)] for e in arole)
                + sum(pref[e][max(0, int(C[e]) - SB)] for e in brole))

    def err(SA, SB):
        return float(np.sqrt(K2_CAL * mass(SA, SB) + C0SQ_CAL))

    def stream_cost(S, weak):
        per_chunk = WEAK_NS if weak else STRONG_NS
        n_mm_floor = 128 * 31.0 if weak else 256 * 31.0
        return sum(max(per_chunk(tn), n_mm_floor)
                   for (_, tn) in _chunks_of(max(S, 1)))

    def plan_cost(SA, SB):
        WA = _round16(max(int(C[arole].max()) - SA, 16))
        WB = _round16(max(int(C[brole].max()) - SB, 16))
        return (stream_cost(SA, False) + stream_cost(SB, False)
                + stream_cost(WA, True) + stream_cost(WB, True))

    maxA, maxB = int(C[arole].max()), int(C[brole].max())
    best = None
    for SA in range(TOK_CHUNK, max(TOK_CHUNK - 129, maxA - TOK_CHUNK - 1), -16):
        for SB in range(TOK_CHUNK, max(TOK_CHUNK - 129, maxB - TOK_CHUNK - 1), -16):
            if maxA - SA > TOK_CHUNK or maxB - SB > TOK_CHUNK:
                continue  # weak stream must stay a single DR chunk
            if err(SA, SB) > ERR_CAP:
                continue
            c = plan_cost(SA, SB)
            if best is None or c < best[0]:
                best = (c, SA, SB)
    if best is None:
        # error-budget safety valve: nothing feasible under the cap, so
        # run every pair in bf16 (correct, just slower: tail chunks)
        SA, SB = maxA, maxB
    else:
        _, SA, SB = best

    split = []
    for e, (tok, cw) in enumerate(routes):
        cap = SA if e in arole else SB
        w_e = max(0, len(tok) - cap)
        idx = np.argsort(cw, kind="stable")
        wk = np.zeros(len(tok), dtype=bool)
        wk[idx[:w_e]] = True
        split.append(((tok[~wk], cw[~wk]), (tok[wk], cw[wk])))
    pairs = [(a, b) for a, b in zip(arole, brole)]
    return split, pairs


def _round16(n):
    return max(16, (n + 15) // 16 * 16)


def _pack_x_bf16(hs, tok, C, first=None):
    bf16 = ml_dtypes.bfloat16
    xt = np.zeros((H, C), dtype=bf16)
    if len(tok):
        xt[:, :len(tok)] = hs[tok].T.astype(bf16)
    return _pack_groups(xt, KT1, _chunks_of(C, first=first))


def _pack_x_f8(hs, tok, C):
    f8 = ml_dtypes.float8_e4m3
    xt = np.zeros((H, C), dtype=f8)
    if len(tok):
        xt[:, :len(tok)] = np.clip(hs[tok].T, -240, 240).astype(f8)
    return _pack_groups(xt, KT1, _chunks_of(C))


def _q8(w):
    return np.clip(w * W8SCALE, -240, 240).astype(ml_dtypes.float8_e4m3)


def kernel(hidden_states, expert_weights, W1, b1, W2, b2, selected_experts):
    from concourse.bass_utils import run_bass_kernel_spmd

    hs = np.asarray(hidden_states)
    out_dtype = hs.dtype
    hs = hs.astype(np.float32)
    W1 = np.asarray(W1).astype(np.float32)
    b1 = np.asarray(b1).astype(np.float32)
    W2 = np.asarray(W2).astype(np.float32)
    b2 = np.asarray(b2).astype(np.float32)

    T = hs.shape[0]
    assert hs.shape[1] == H and W1.shape == (E, H, F) and W2.shape == (E, F, H)

    routes = _route(expert_weights, selected_experts)
    split, pairs = _plan_split(routes)
    s_cnt = np.array([len(s[0][0]) for s in split])
    w_cnt = np.array([len(s[1][0]) for s in split])

    SA = max(PART, int(max(s_cnt[a] for a, _ in pairs)))
    SB = max(PART, int(max(s_cnt[b] for _, b in pairs)))
    WA = _round16(int(max(w_cnt[a] for a, _ in pairs)))
    WB = _round16(int(max(w_cnt[b] for _, b in pairs)))

    key = (SA, SB, WA, WB)
    if key not in _PROGRAM_CACHE:
        _PROGRAM_CACHE[key] = _build_program(*key)
    nc = _PROGRAM_CACHE[key]

    bf16 = ml_dtypes.bfloat16
    w1_groups = _w1_groups()
    w2_groups = _w2_groups()

    w1h16 = {}
    w2h16 = {}
    w1h8 = {}
    w2h8 = {}
    b1h = {}
    for e in set(e for p in pairs for e in p):
        for half in (0, 1):
            c0, c1 = half * FH, (half + 1) * FH
            w1e = W1[e][:, c0:c1]
            w2e = W2[e][c0:c1, :]
            w1h16[(e, half)] = _pack_groups(w1e.astype(bf16), KT1, w1_groups)
            w2h16[(e, half)] = _pack_groups(w2e.astype(bf16), KT2, w2_groups)
            w1h8[(e, half)] = _pack_3d(_q8(w1e), KT1)
            w2h8[(e, half)] = _pack_3d(_q8(w2e), KT2)
            b1h[(e, half)] = np.ascontiguousarray(
                b1[e][c0:c1].reshape(MT1, PART).T)

    xs_cache = {}
    xw_cache = {}
    in_maps = []
    for (a, b) in pairs:
        if a not in xs_cache:
            xs_cache[a] = _pack_x_bf16(hs, split[a][0][0], SA,
                                       first=FIRST_CHUNK)
            xw_cache[a] = _pack_x_f8(hs, split[a][1][0], WA)
        if b not in xs_cache:
            xs_cache[b] = _pack_x_bf16(hs, split[b][0][0], SB)
            xw_cache[b] = _pack_x_f8(hs, split[b][1][0], WB)
        for half in (0, 1):
            in_maps.append({
                "xas": xs_cache[a], "xaw": xw_cache[a],
                "xbs": xs_cache[b], "xbw": xw_cache[b],
                "w1a16": w1h16[(a, half)], "w2a16": w2h16[(a, half)],
                "w1a8": w1h8[(a, half)], "w2a8": w2h8[(a, half)],
                "w1b16": w1h16[(b, 1 - half)], "w2b16": w2h16[(b, 1 - half)],
                "w1b8": w1h8[(b, 1 - half)], "w2b8": w2h8[(b, 1 - half)],
                "b1ab": np.ascontiguousarray(np.concatenate(
                    [b1h[(a, half)], b1h[(b, 1 - half)]], axis=1)),
            })

    res = run_bass_kernel_spmd(nc, in_maps, core_ids=list(range(N_CORES)))

    def unfold(buf, cnt):
        # [PART, MT2*S] partition-major device layout -> [H, cnt]
        S = buf.shape[1] // MT2
        return (buf.reshape(PART, MT2, S).transpose(1, 0, 2)
                .reshape(H, S)[:, :cnt].astype(np.float32))

    out = np.zeros((T, H), dtype=np.float32)
    for pi, (a, b) in enumerate(pairs):
        r0 = res.results[2 * pi]
        r1 = res.results[2 * pi + 1]
        for e, skey, wkey in ((a, "yas", "yaw"), (b, "ybs", "ybw")):
            (tok_s, cw_s), (tok_w, cw_w) = split[e]
            if len(tok_s):
                yt = (unfold(r0[skey], len(tok_s))
                      + unfold(r1[skey], len(tok_s)))
                out[tok_s] += cw_s[:, None] * (yt.T + b2[e][None, :])
            if len(tok_w):
                yt = (unfold(r0[wkey], len(tok_w))
                      + unfold(r1[wkey], len(tok_w)))
                out[tok_w] += cw_w[:, None] * (yt.T / W8SCALE
                                               + b2[e][None, :])
    return out.astype(out_dtype)

